# revision 1
# baseline (speedup 1.0000x reference)
"""Multi-head causal attention (B=4, S=2048, D=2048, H=16) on 8 trn2 cores.

Sharding: core c handles batch b = c//2 and head-group g = c%2 (8 heads).
Each core computes q/k/v projections for its heads, causal flash-style
attention, and a partial out_proj over its dv-slice. Host sums the two
partials per batch.

Device pipeline (per core), all matmuls in float32r (TF32-like, 1 cyc/row):
  phase 1a: qkT[e, s] = WqkT-chunks.T @ xT   (e: 8 q-heads then 8 k-heads)
  phase 1b: v[s, ev] = xT-chunks.T @ WvT, scaled by exp(alibi_bias[h, k])
            during PSUM evacuation (folds the ALiBi bias into softmax via
            exp(s + b) = exp(s) * exp(b))
  phase 2 per (head, 512-query-block): scoresT[k, q] = kT-chunk.T @ qT,
     two 512-wide key-chunks into one [128, 1024] PSUM tile
     -> one wide ACT exp -> GPSIMD affine_select zeroes the causal
        upper-staircase on diagonal chunks
     -> sumexp[*, q] += ebias-col-broadcast.T @ expT   (all-partition rows)
     -> attnT[dv, q] += v'-chunk.T @ expT
     -> attnT *= 1/sumexp  (DVE reciprocal + mul)
  phase 3: O[s, e] = attnT-chunks.T @ out_projT  (partial over dv-slice)

The scoresT (keys-on-partitions) layout makes every matmul operand natural:
no transposes anywhere, and softmax sums reduce over the partition axis via
a single matmul whose stationary operand is exp(bias) broadcast along free.
"""
import os
import sys
import types

if "/opt/trn_rl_repo" not in sys.path:
    sys.path.insert(0, "/opt/trn_rl_repo")

import numpy as np

B, S, D, H = 4, 2048, 2048, 16
HD = D // H          # 128 head dim
HPC = H // 2         # 8 heads per core
EV = HPC * HD        # 1024 dv-slice per core
NKC = S // 128       # 16 key chunks
NSC = S // 512       # 4 query super-blocks
NDC = D // 128       # 16 contraction chunks

_NC_CACHE = {}
LAST_EXEC_NS = None
LAST_PER_CORE_NS = None


def _install_ntff_hook():
    try:
        import antenv
        if "antenv.axon_hooks" in sys.modules:
            return
        mod = types.ModuleType("antenv.axon_hooks")
        state = {"hook": None}
        mod.set_axon_ntff_profile_hook = lambda h: state.__setitem__("hook", h)
        mod.get_axon_ntff_profile_hook = lambda: state["hook"]
        sys.modules["antenv.axon_hooks"] = mod
        antenv.axon_hooks = mod
        from trn_agent_boot.trn_boot import _ntff_profile_via_ctypes
        mod.set_axon_ntff_profile_hook(
            _ntff_profile_via_ctypes("/opt/axon/libaxon_pjrt.so"))
    except Exception:
        pass


def _build_nc():
    import concourse.bacc as bacc
    import concourse.mybir as mybir
    import concourse.tile as tile

    F32 = mybir.dt.float32
    F32R = mybir.dt.float32r
    EXP = mybir.ActivationFunctionType.Exp

    nc = bacc.Bacc()
    xt = nc.dram_tensor("xt", [D, S], F32R, kind="ExternalInput")
    wqkt = nc.dram_tensor("wqkt", [D, 2 * EV], F32R, kind="ExternalInput")
    wvt = nc.dram_tensor("wvt", [D, EV], F32R, kind="ExternalInput")
    ptt = nc.dram_tensor("ptt", [EV, D], F32R, kind="ExternalInput")
    ebias_r = nc.dram_tensor("ebias_r", [128, HPC * NKC], F32R,
                             kind="ExternalInput")
    ebias_f = nc.dram_tensor("ebias_f", [128, HPC * NKC], F32,
                             kind="ExternalInput")
    out = nc.dram_tensor("o", [S, D], F32, kind="ExternalOutput")

    with tile.TileContext(nc) as tc:
        with (
            tc.tile_pool(name="consts", bufs=1) as cp,
            tc.tile_pool(name="dram", bufs=1, space="DRAM") as dp,
        ):
            ebr_t = cp.tile([128, HPC * NKC], F32R, tag="ebr")
            ebf_t = cp.tile([128, HPC * NKC], F32, tag="ebf")
            nc.sync.dma_start(ebr_t[:], ebias_r[:])
            nc.sync.dma_start(ebf_t[:], ebias_f[:])

            qkT_d = dp.tile([2 * EV, S], F32R, tag="qkT_d")
            v_d = dp.tile([S, EV], F32R, tag="v_d")

            # ---------------- phase 1: projections ----------------
            with tc.tile_pool(name="xp", bufs=1) as xp:
                x_tiles = []
                for dc in range(NDC):
                    x_t = xp.tile([128, S], F32R, tag=f"x{dc}", name=f"x{dc}")
                    nc.sync.dma_start(x_t[:], xt[128 * dc:128 * (dc + 1), :])
                    x_tiles.append(x_t)

                # phase 1a: qkT (16 e-chunks x 4 s-cols, accumulate 16 d)
                with (
                    tc.tile_pool(name="wqk", bufs=2) as wp,
                    tc.tile_pool(name="st1", bufs=4) as sp,
                    tc.tile_pool(name="ps1", bufs=4, space="PSUM") as pp,
                ):
                    for ec in range(16):
                        w_t = wp.tile([128, NDC, 128], F32R, tag="w")
                        for dc in range(NDC):
                            nc.sync.dma_start(
                                w_t[:, dc],
                                wqkt[128 * dc:128 * (dc + 1),
                                     128 * ec:128 * (ec + 1)])
                        for sc in range(NSC):
                            ps = pp.tile([128, 512], F32, tag="p")
                            for dc in range(NDC):
                                nc.tensor.matmul(
                                    ps[:], w_t[:, dc],
                                    x_tiles[dc][:, 512 * sc:512 * (sc + 1)],
                                    start=(dc == 0), stop=(dc == NDC - 1))
                            st = sp.tile([128, 512], F32R, tag="st")
                            nc.any.tensor_copy(st[:], ps[:])
                            nc.sync.dma_start(
                                qkT_d[128 * ec:128 * (ec + 1),
                                      512 * sc:512 * (sc + 1)], st[:])

                # phase 1b: v (2 ev-cols x 16 s-chunks, accumulate 16 d);
                # evacuation applies the per-(head, key) exp(bias) scale.
                with (
                    tc.tile_pool(name="wv", bufs=1) as wvp,
                    tc.tile_pool(name="st2", bufs=4) as sp2,
                    tc.tile_pool(name="ps2", bufs=4, space="PSUM") as pp2,
                ):
                    for evc in range(EV // 512):
                        wv_t = wvp.tile([128, NDC, 512], F32R, tag="wv")
                        for dc in range(NDC):
                            nc.sync.dma_start(
                                wv_t[:, dc],
                                wvt[128 * dc:128 * (dc + 1),
                                    512 * evc:512 * (evc + 1)])
                        for sc16 in range(NKC):
                            ps = pp2.tile([128, 512], F32, tag="p")
                            for dc in range(NDC):
                                nc.tensor.matmul(
                                    ps[:],
                                    x_tiles[dc][:, 128 * sc16:128 * (sc16 + 1)],
                                    wv_t[:, dc],
                                    start=(dc == 0), stop=(dc == NDC - 1))
                            st = sp2.tile([128, 512], F32R, tag="st")
                            for hl in range(4):
                                h = 4 * evc + hl
                                col = h * NKC + sc16
                                nc.vector.tensor_scalar(
                                    out=st[:, 128 * hl:128 * (hl + 1)],
                                    in0=ps[:, 128 * hl:128 * (hl + 1)],
                                    scalar1=ebf_t[:, col:col + 1],
                                    scalar2=None,
                                    op0=mybir.AluOpType.mult)
                            nc.sync.dma_start(
                                v_d[128 * sc16:128 * (sc16 + 1),
                                    512 * evc:512 * (evc + 1)], st[:])

            # ---------------- phase 2: attention ----------------
            with tc.tile_pool(name="attn", bufs=1) as ap:
                attn_sb = []
                for h in range(HPC):
                    a_t = ap.tile([128, S], F32R, tag=f"a{h}", name=f"a{h}")
                    attn_sb.append(a_t)

                with (
                    tc.tile_pool(name="head", bufs=2) as hp,
                    tc.tile_pool(name="expt", bufs=4) as ep,
                    tc.tile_pool(name="wk2", bufs=3) as wk,
                    tc.tile_pool(name="ps_acc", bufs=2, space="PSUM") as pa,
                    tc.tile_pool(name="ps_sc", bufs=2, space="PSUM") as pc,
                ):
                    for h in range(HPC):
                        qt_h = hp.tile([128, S], F32R, tag="qt")
                        kt_h = hp.tile([128, S], F32R, tag="kt")
                        v_h = hp.tile([128, NKC, HD], F32R, tag="v")
                        nc.sync.dma_start(
                            qt_h[:], qkT_d[128 * h:128 * (h + 1), :])
                        nc.sync.dma_start(
                            kt_h[:], qkT_d[EV + 128 * h:EV + 128 * (h + 1), :])
                        for sc16 in range(NKC):
                            nc.sync.dma_start(
                                v_h[:, sc16],
                                v_d[128 * sc16:128 * (sc16 + 1),
                                    HD * h:HD * (h + 1)])
                        for qsb in range(NSC):
                            npair = 2 * qsb + 2
                            nkc = 2 * npair
                            at_ps = pa.tile([128, 512], F32, tag="at")
                            se_ps = pa.tile([128, 512], F32, tag="se")

                            def se_at(e_t, kp):
                                for half in range(2):
                                    kc = 2 * kp + half
                                    col = h * NKC + kc
                                    nc.tensor.matmul(
                                        se_ps[:],
                                        ebr_t[:, col:col + 1]
                                        .broadcast_to([128, 128]),
                                        e_t[:, 512 * half:512 * (half + 1)],
                                        start=(kc == 0), stop=(kc == nkc - 1))
                                    nc.tensor.matmul(
                                        at_ps[:], v_h[:, kc],
                                        e_t[:, 512 * half:512 * (half + 1)],
                                        start=(kc == 0), stop=(kc == nkc - 1))

                            prev = None
                            for kp in range(npair):
                                sc_ps = pc.tile([128, 1024], F32, tag="sc")
                                for half in range(2):
                                    kc = 2 * kp + half
                                    nc.tensor.matmul(
                                        sc_ps[:, 512 * half:512 * (half + 1)],
                                        kt_h[:, 128 * kc:128 * (kc + 1)],
                                        qt_h[:, 512 * qsb:512 * (qsb + 1)],
                                        start=True, stop=True)
                                e_t = ep.tile([128, 1024], F32R, tag="e")
                                nc.scalar.activation(e_t[:], sc_ps[:], EXP,
                                                     bias=0.0, scale=1.0)
                                for half in range(2):
                                    kc = 2 * kp + half
                                    p = kc - 4 * qsb
                                    if p >= 0:
                                        # zero where q < k:
                                        # keep j >= i + 128p, else fill 0
                                        nc.gpsimd.affine_select(
                                            out=e_t[:, 512 * half:
                                                    512 * (half + 1)],
                                            in_=e_t[:, 512 * half:
                                                    512 * (half + 1)],
                                            compare_op=mybir.AluOpType.is_ge,
                                            fill=0.0,
                                            base=-128 * p,
                                            pattern=[[1, 512]],
                                            channel_multiplier=-1)
                                if prev is not None:
                                    se_at(*prev)
                                prev = (e_t, kp)
                            se_at(*prev)
                            recip = wk.tile([128, 512], F32, tag="recip")
                            nc.vector.reciprocal(recip[:], se_ps[:])
                            nc.vector.tensor_mul(
                                attn_sb[h][:, 512 * qsb:512 * (qsb + 1)],
                                at_ps[:], recip[:])

                # ---------------- phase 3: out_proj partial ----------------
                with (
                    tc.tile_pool(name="pt", bufs=1) as ptp,
                    tc.tile_pool(name="st3", bufs=4) as sp3,
                    tc.tile_pool(name="ps3", bufs=4, space="PSUM") as pp3,
                ):
                    pt_t = ptp.tile([128, HPC, NSC, 512], F32R, tag="pt")
                    for dvc in range(HPC):
                        for ec in range(NSC):
                            nc.sync.dma_start(
                                pt_t[:, dvc, ec],
                                ptt[128 * dvc:128 * (dvc + 1),
                                    512 * ec:512 * (ec + 1)])
                    for sc16 in range(NKC):
                        for ec in range(NSC):
                            ps = pp3.tile([128, 512], F32, tag="p")
                            for dvc in range(HPC):
                                nc.tensor.matmul(
                                    ps[:],
                                    attn_sb[dvc][:, 128 * sc16:128 * (sc16 + 1)],
                                    pt_t[:, dvc, ec],
                                    start=(dvc == 0), stop=(dvc == HPC - 1))
                            st = sp3.tile([128, 512], F32, tag="st")
                            nc.any.tensor_copy(st[:], ps[:])
                            nc.sync.dma_start(
                                out[128 * sc16:128 * (sc16 + 1),
                                    512 * ec:512 * (ec + 1)], st[:])
    nc.finalize()
    return nc


def _get_nc():
    if "nc" not in _NC_CACHE:
        _NC_CACHE["nc"] = _build_nc()
    return _NC_CACHE["nc"]


def _prepare_core_inputs(x, Wqkv_w, out_proj_w, attn_bias):
    scale = 1.0 / np.sqrt(HD)
    in_maps = []
    for c in range(8):
        b, g = c // 2, c % 2
        hlo, hhi = g * EV, (g + 1) * EV
        wq = Wqkv_w[hlo:hhi] * scale            # [1024, D]
        wk = Wqkv_w[D + hlo:D + hhi]            # [1024, D]
        wv = Wqkv_w[2 * D + hlo:2 * D + hhi]    # [1024, D]
        wqkt = np.ascontiguousarray(
            np.concatenate([wq, wk], axis=0).T)  # [D, 2048]
        wvt = np.ascontiguousarray(wv.T)         # [D, 1024]
        ptt = np.ascontiguousarray(out_proj_w[:, hlo:hhi].T)  # [1024, D]
        xt = np.ascontiguousarray(x[b].T)        # [D, S]
        # ebias[i, h*16+kc] = exp(attn_bias[0, g*8+h, 0, kc*128+i])
        bias_g = attn_bias[0, g * HPC:(g + 1) * HPC, 0, :]     # [8, S]
        ebias = np.exp(np.ascontiguousarray(
            bias_g.reshape(HPC, NKC, 128).transpose(2, 0, 1)
            .reshape(128, HPC * NKC)).astype(np.float64)).astype(np.float32)
        in_maps.append({
            "xt": xt, "wqkt": wqkt, "wvt": wvt, "ptt": ptt,
            "ebias_r": ebias, "ebias_f": ebias,
        })
    return in_maps


def kernel(x, Wqkv_w, out_proj_w, attn_bias, key_padding_mask=None):
    """Full inputs in, full [B, S, D] float32 output out.

    key_padding_mask is all-True for this problem spec and is ignored.
    """
    global LAST_EXEC_NS, LAST_PER_CORE_NS
    from concourse.bass_utils import run_bass_kernel_spmd

    x = np.asarray(x, dtype=np.float32)
    Wqkv_w = np.asarray(Wqkv_w, dtype=np.float32)
    out_proj_w = np.asarray(out_proj_w, dtype=np.float32)
    attn_bias = np.asarray(attn_bias, dtype=np.float32)

    trace = bool(int(os.environ.get("KERNEL_TRACE", "0")))
    if trace:
        _install_ntff_hook()

    nc = _get_nc()
    in_maps = _prepare_core_inputs(x, Wqkv_w, out_proj_w, attn_bias)
    kwargs = {}
    if trace:
        kwargs.update(trace=True, trace_cores=list(range(8)))
    res = run_bass_kernel_spmd(nc, in_maps, core_ids=list(range(8)), **kwargs)
    LAST_EXEC_NS = res.exec_time_ns
    LAST_PER_CORE_NS = res.mean_exec_time_ns

    out = np.empty((B, S, D), dtype=np.float32)
    for b in range(B):
        out[b] = res.results[2 * b]["o"] + res.results[2 * b + 1]["o"]
    return out



# revision 8
# speedup vs baseline: 1.1953x; 1.1953x over previous
"""Multi-head causal attention (B=4, S=2048, D=2048, H=16) on 8 trn2 cores.

Sharding: core c handles batch b = c//2 and head-group g = c%2 (8 heads).
Each core computes q/k/v projections for its heads, causal attention, and a
partial out_proj over its dv-slice. Host sums the two partials per batch.

v2: all matmuls in bfloat16 (same 1 cyc/row as f32r but half the bytes),
everything SBUF-resident (no DRAM round trip for q/k/v), input DMAs ordered
so the first projection chains consume x chunks as they stream in, 256-wide
query blocks (56.25% causal coverage), and out_proj interleaved per query
block.

Device pipeline (per core):
  phase 1b: v[s, ev] = xT-chunks.T @ WvT, scaled by exp(alibi_bias[h, k])
            during PSUM evacuation (folds ALiBi into softmax via
            exp(s + b) = exp(s) * exp(b)). First 8 chains are emitted
            dc-outer so they consume x chunks as the DMA stream lands.
  phase 1a: qkT[e, s] = WqkT-chunks.T @ xT   (e: 8 q-heads then 8 k-heads)
  phase 2 per (256-query block, head): scoresT[k, q] = kT-chunk.T @ qT,
     four 128-key chunks into one [128, 1024] PSUM tile
     -> one wide ACT exp -> GPSIMD affine_select zeroes the causal
        staircase on the two diagonal chunks
     -> sumexp[*, q] += ebias-col-broadcast.T @ expT
     -> attnT[dv, q] += v'-chunk.T @ expT
     -> attnT *= 1/sumexp  (DVE reciprocal + mul)
  phase 3 (interleaved, one query block behind): O[s, e] partial
     = attnT-chunks.T @ out_projT over this core's dv-slice.
"""
import os
import sys
import types
from collections import deque

if "/opt/trn_rl_repo" not in sys.path:
    sys.path.insert(0, "/opt/trn_rl_repo")

import numpy as np

B, S, D, H = 4, 2048, 2048, 16
HD = D // H          # 128 head dim
HPC = H // 2         # 8 heads per core
EV = HPC * HD        # 1024 dv-slice per core
NKC = S // 128       # 16 key chunks
NDC = D // 128       # 16 contraction chunks
QW = 256             # query block width
NQB = S // QW        # 8 query blocks

_NC_CACHE = {}
LAST_EXEC_NS = None
LAST_PER_CORE_NS = None


def _install_ntff_hook():
    try:
        import antenv
        if "antenv.axon_hooks" in sys.modules:
            return
        mod = types.ModuleType("antenv.axon_hooks")
        state = {"hook": None}
        mod.set_axon_ntff_profile_hook = lambda h: state.__setitem__("hook", h)
        mod.get_axon_ntff_profile_hook = lambda: state["hook"]
        sys.modules["antenv.axon_hooks"] = mod
        antenv.axon_hooks = mod
        from trn_agent_boot.trn_boot import _ntff_profile_via_ctypes
        mod.set_axon_ntff_profile_hook(
            _ntff_profile_via_ctypes("/opt/axon/libaxon_pjrt.so"))
    except Exception:
        pass


def _build_nc():
    import concourse.bacc as bacc
    import concourse.mybir as mybir
    import concourse.tile as tile

    F32 = mybir.dt.float32
    BF16 = mybir.dt.bfloat16
    EXP = mybir.ActivationFunctionType.Exp
    MULT = mybir.AluOpType.mult
    GE = mybir.AluOpType.is_ge

    nc = bacc.Bacc()
    # xt[p, dc, s] = x[b, s, 128*dc+p]
    xt = nc.dram_tensor("xt", [128, NDC, S], BF16, kind="ExternalInput")
    # wqk[p, ec, dc, e] = Wqk_scaled[128*ec+e, 128*dc+p]
    wqk = nc.dram_tensor("wqk", [128, 16, NDC, 128], BF16,
                         kind="ExternalInput")
    # wv[p, evc, dc, c] = Wv[512*evc+c, 128*dc+p]
    wv = nc.dram_tensor("wv", [128, 2, NDC, 512], BF16, kind="ExternalInput")
    # ptt[p, dvc, e] = out_proj_w[e, 128*dvc+p]  (within this core's slice)
    ptt = nc.dram_tensor("ptt", [128, HPC, D], BF16, kind="ExternalInput")
    # ebias[i, h*16+kc] = exp(attn_bias[h, kc*128+i])
    ebias_r = nc.dram_tensor("ebias_r", [128, HPC * NKC], BF16,
                             kind="ExternalInput")
    ebias_f = nc.dram_tensor("ebias_f", [128, HPC * NKC], F32,
                             kind="ExternalInput")
    out = nc.dram_tensor("o", [S, D], F32, kind="ExternalOutput")

    with tile.TileContext(nc) as tc:
        with (
            tc.tile_pool(name="consts", bufs=1) as cp,
            tc.tile_pool(name="qk", bufs=1) as qkp,
            tc.tile_pool(name="vv", bufs=1) as vp,
        ):
            ebr_t = cp.tile([128, HPC * NKC], BF16, tag="ebr", name="ebr")
            ebf_t = cp.tile([128, HPC * NKC], F32, tag="ebf", name="ebf")
            nc.sync.dma_start(ebr_t[:], ebias_r[:])
            nc.sync.dma_start(ebf_t[:], ebias_f[:])

            v_tiles = [vp.tile([128, EV], BF16, tag=f"v{sc}", name=f"v{sc}")
                       for sc in range(NKC)]
            qk_tiles = [qkp.tile([128, S], BF16, tag=f"qk{ec}",
                                 name=f"qk{ec}")
                        for ec in range(16)]

            # ---------------- phase 1: projections ----------------
            with (
                tc.tile_pool(name="xp", bufs=1) as xp,
                tc.tile_pool(name="ps1", bufs=8, space="PSUM") as pp,
            ):
                # phase 1b (v) first: its first 8 chains absorb the x-load
                # latency chunk by chunk.
                with tc.tile_pool(name="wvp", bufs=2) as wvp:
                    wv_t0 = wvp.tile([128, NDC, 512], BF16, tag="wv",
                                     name="wv_t0")
                    x_tiles = []
                    for dc in range(NDC):
                        nc.sync.dma_start(wv_t0[:, dc], wv[:, 0, dc])
                        x_t = xp.tile([128, S], BF16, tag=f"x{dc}",
                                      name=f"x{dc}")
                        nc.sync.dma_start(x_t[:], xt[:, dc])
                        x_tiles.append(x_t)
                    wv_t1 = wvp.tile([128, NDC, 512], BF16, tag="wv",
                                     name="wv_t1")
                    for dc in range(NDC):
                        nc.sync.dma_start(wv_t1[:, dc], wv[:, 1, dc])

                    def v_evac(ps, evc, sc):
                        for hl in range(4):
                            h = 4 * evc + hl
                            col = h * NKC + sc
                            nc.vector.tensor_scalar(
                                out=v_tiles[sc][:, 512 * evc + 128 * hl:
                                                512 * evc + 128 * (hl + 1)],
                                in0=ps[:, 128 * hl:128 * (hl + 1)],
                                scalar1=ebf_t[:, col:col + 1],
                                scalar2=None,
                                op0=MULT)

                    # wave 0: 8 chains (evc=0, sc 0..7), dc-outer so each
                    # arriving x chunk unlocks the next step of all chains.
                    ps_w = [pp.tile([128, 512], F32, tag="p", name=f"pw{i}")
                            for i in range(8)]
                    for dc in range(NDC):
                        for sc in range(8):
                            nc.tensor.matmul(
                                ps_w[sc][:],
                                x_tiles[dc][:, 128 * sc:128 * (sc + 1)],
                                wv_t0[:, dc],
                                start=(dc == 0), stop=(dc == NDC - 1))
                    for sc in range(8):
                        v_evac(ps_w[sc], 0, sc)
                    # remaining v chains at full speed
                    for evc, wv_t, scs in ((0, wv_t0, range(8, 16)),
                                           (1, wv_t1, range(16))):
                        for sc in scs:
                            ps = pp.tile([128, 512], F32, tag="p", name="pv")
                            for dc in range(NDC):
                                nc.tensor.matmul(
                                    ps[:],
                                    x_tiles[dc][:, 128 * sc:128 * (sc + 1)],
                                    wv_t[:, dc],
                                    start=(dc == 0), stop=(dc == NDC - 1))
                            v_evac(ps, evc, sc)

                # phase 1a: qkT (16 e-chunks x 4 s-cols, accumulate 16 d)
                with tc.tile_pool(name="wp", bufs=3) as wp:
                    for ec in range(16):
                        w_t = wp.tile([128, NDC, 128], BF16, tag="w",
                                      name="w_t")
                        nc.sync.dma_start(w_t[:], wqk[:, ec])
                        for sc4 in range(4):
                            ps = pp.tile([128, 512], F32, tag="p", name="pq")
                            for dc in range(NDC):
                                nc.tensor.matmul(
                                    ps[:], w_t[:, dc],
                                    x_tiles[dc][:, 512 * sc4:512 * (sc4 + 1)],
                                    start=(dc == 0), stop=(dc == NDC - 1))
                            nc.scalar.copy(
                                out=qk_tiles[ec][:, 512 * sc4:
                                                 512 * (sc4 + 1)],
                                in_=ps[:])

            # ---------------- phase 2 + 3: attention + out_proj ----------
            with (
                tc.tile_pool(name="attn", bufs=1) as ap,
                tc.tile_pool(name="ptp", bufs=1) as ptp,
                tc.tile_pool(name="expt", bufs=3) as ep,
                tc.tile_pool(name="rcp", bufs=2) as rp,
                tc.tile_pool(name="ost", bufs=3) as stp,
                tc.tile_pool(name="ps_sc", bufs=2, space="PSUM") as pcp,
                tc.tile_pool(name="ps_acc", bufs=2, space="PSUM") as pap,
            ):
                a_tiles = [ap.tile([128, S], BF16, tag=f"a{h}", name=f"a{h}")
                           for h in range(HPC)]
                pt_tiles = []
                for dvc in range(HPC):
                    pt_t = ptp.tile([128, D], BF16, tag=f"pt{dvc}",
                                    name=f"pt{dvc}")
                    nc.sync.dma_start(pt_t[:], ptt[:, dvc])
                    pt_tiles.append(pt_t)

                pending = deque()

                def flush(keep=0):
                    while len(pending) > keep:
                        pending.popleft()()

                def make_se_at(e_t, quad, h, at_ps, se_ps, nkc):
                    def run():
                        for i, kc in enumerate(quad):
                            col = h * NKC + kc
                            st = dict(start=(kc == 0), stop=(kc == nkc - 1))
                            nc.tensor.matmul(
                                se_ps[:],
                                ebr_t[:, col:col + 1]
                                .broadcast_to([128, 128]),
                                e_t[:, 256 * i:256 * (i + 1)], **st)
                            nc.tensor.matmul(
                                at_ps[:],
                                v_tiles[kc][:, 128 * h:128 * (h + 1)],
                                e_t[:, 256 * i:256 * (i + 1)], **st)
                    return run

                def make_fin(h, qb, at_ps, se_ps):
                    def run():
                        rc = rp.tile([128, QW], F32, tag="rc", name="rc")
                        nc.vector.reciprocal(rc[:], se_ps[:])
                        nc.vector.tensor_mul(
                            a_tiles[h][:, QW * qb:QW * (qb + 1)],
                            at_ps[:], rc[:])
                    return run

                def out_proj(qb):
                    for scl in range(2):
                        sc16 = 2 * qb + scl
                        for ec in range(4):
                            po_f = pcp.tile([128, 1024], F32, tag="sc",
                                            name="po")
                            po_t = po_f[:, 0:512]
                            for dvc in range(HPC):
                                nc.tensor.matmul(
                                    po_t,
                                    a_tiles[dvc][:, 128 * sc16:
                                                 128 * (sc16 + 1)],
                                    pt_tiles[dvc][:, 512 * ec:
                                                  512 * (ec + 1)],
                                    start=(dvc == 0), stop=(dvc == HPC - 1))
                            st = stp.tile([128, 512], F32, tag="st",
                                          name="st")
                            nc.vector.tensor_copy(st[:], po_t)
                            nc.sync.dma_start(
                                out[128 * sc16:128 * (sc16 + 1),
                                    512 * ec:512 * (ec + 1)], st[:])

                for qb in range(NQB):
                    nkc = 2 * qb + 2
                    quads = [list(range(q0, min(q0 + 4, nkc)))
                             for q0 in range(0, nkc, 4)]
                    for h in range(HPC):
                        at_ps = pap.tile([128, QW], F32, tag="at", name="at")
                        se_ps = pap.tile([128, QW], F32, tag="se", name="se")
                        for quad in quads:
                            sc_ps = pcp.tile([128, 1024], F32, tag="sc",
                                             name="sc")
                            for i, kc in enumerate(quad):
                                nc.tensor.matmul(
                                    sc_ps[:, 256 * i:256 * (i + 1)],
                                    qk_tiles[HPC + h][:, 128 * kc:
                                                      128 * (kc + 1)],
                                    qk_tiles[h][:, QW * qb:QW * (qb + 1)],
                                    start=True, stop=True)
                            w = 256 * len(quad)
                            e_t = ep.tile([128, 1024], BF16, tag="e",
                                          name="e")
                            nc.scalar.activation(e_t[:, :w], sc_ps[:, :w],
                                                 EXP, bias=0.0, scale=1.0)
                            for i, kc in enumerate(quad):
                                p = kc - 2 * qb
                                if p >= 0:
                                    # zero where q < k: keep j >= i + 128p
                                    nc.gpsimd.affine_select(
                                        out=e_t[:, 256 * i:256 * (i + 1)],
                                        in_=e_t[:, 256 * i:256 * (i + 1)],
                                        compare_op=GE,
                                        fill=0.0,
                                        base=-128 * p,
                                        pattern=[[1, QW]],
                                        channel_multiplier=-1)
                            pending.append(
                                make_se_at(e_t, quad, h, at_ps, se_ps, nkc))
                            flush(keep=1)
                        pending.append(make_fin(h, qb, at_ps, se_ps))
                    if qb > 0:
                        out_proj(qb - 1)
                flush()
                out_proj(NQB - 1)
    nc.finalize()
    return nc


def _get_nc():
    if "nc" not in _NC_CACHE:
        _NC_CACHE["nc"] = _build_nc()
    return _NC_CACHE["nc"]


def _prepare_core_inputs(x, Wqkv_w, out_proj_w, attn_bias):
    import ml_dtypes
    BF = ml_dtypes.bfloat16
    scale = 1.0 / np.sqrt(HD)
    in_maps = []
    for c in range(8):
        b, g = c // 2, c % 2
        hlo, hhi = g * EV, (g + 1) * EV
        wq = Wqkv_w[hlo:hhi] * scale            # [1024, D]
        wk = Wqkv_w[D + hlo:D + hhi]            # [1024, D]
        wvm = Wqkv_w[2 * D + hlo:2 * D + hhi]   # [1024, D]
        wqk_m = np.concatenate([wq, wk], axis=0)  # [2048, D]
        # wqk[p, ec, dc, e] = wqk_m[128*ec+e, 128*dc+p]
        wqk_t = np.ascontiguousarray(
            wqk_m.reshape(16, 128, NDC, 128).transpose(3, 0, 2, 1)
        ).astype(BF)
        # wv[p, evc, dc, c] = wvm[512*evc+c, 128*dc+p]
        wv_t = np.ascontiguousarray(
            wvm.reshape(2, 512, NDC, 128).transpose(3, 0, 2, 1)).astype(BF)
        # xt[p, dc, s] = x[b, s, 128*dc+p]
        xt = np.ascontiguousarray(
            x[b].reshape(S, NDC, 128).transpose(2, 1, 0)).astype(BF)
        # ptt[p, dvc, e] = out_proj_w[e, hlo + 128*dvc + p]
        pt = out_proj_w[:, hlo:hhi].T            # [1024, D]
        ptt = np.ascontiguousarray(
            pt.reshape(HPC, 128, D).transpose(1, 0, 2)).astype(BF)
        # ebias[i, h*16+kc] = exp(attn_bias[0, g*8+h, 0, kc*128+i])
        bias_g = attn_bias[0, g * HPC:(g + 1) * HPC, 0, :]     # [8, S]
        ebias = np.exp(np.ascontiguousarray(
            bias_g.reshape(HPC, NKC, 128).transpose(2, 0, 1)
            .reshape(128, HPC * NKC)).astype(np.float64)).astype(np.float32)
        in_maps.append({
            "xt": xt, "wqk": wqk_t, "wv": wv_t, "ptt": ptt,
            "ebias_r": ebias.astype(BF), "ebias_f": ebias,
        })
    return in_maps


def kernel(x, Wqkv_w, out_proj_w, attn_bias, key_padding_mask=None):
    """Full inputs in, full [B, S, D] float32 output out.

    key_padding_mask is all-True for this problem spec and is ignored.
    """
    global LAST_EXEC_NS, LAST_PER_CORE_NS
    from concourse.bass_utils import run_bass_kernel_spmd

    x = np.asarray(x, dtype=np.float32)
    Wqkv_w = np.asarray(Wqkv_w, dtype=np.float32)
    out_proj_w = np.asarray(out_proj_w, dtype=np.float32)
    attn_bias = np.asarray(attn_bias, dtype=np.float32)

    trace = bool(int(os.environ.get("KERNEL_TRACE", "0")))
    if trace:
        _install_ntff_hook()

    nc = _get_nc()
    in_maps = _prepare_core_inputs(x, Wqkv_w, out_proj_w, attn_bias)
    kwargs = {}
    if trace:
        kwargs.update(trace=True, trace_cores=list(range(8)))
    res = run_bass_kernel_spmd(nc, in_maps, core_ids=list(range(8)), **kwargs)
    LAST_EXEC_NS = res.exec_time_ns
    LAST_PER_CORE_NS = res.mean_exec_time_ns

    out = np.empty((B, S, D), dtype=np.float32)
    for b in range(B):
        out[b] = res.results[2 * b]["o"] + res.results[2 * b + 1]["o"]
    return out
